# revision 8
# baseline (speedup 1.0000x reference)
"""Trainium2 Bass kernel for nn_CNNMode_Kernal_2 (dense_cnn).

Reference computation (all fp32):
    xp = x.reshape(B, C, L//4, 4)
    conv[b,c,f] = sum_k xp[b,c,f,k] * W1[c,k] + b1[c]          # per-channel Conv1d(1,1,4,4)
    flat = conv.reshape(B, C*F)                                 # channel-major
    h = relu(flat @ W2 + b2)
    out = (h @ W3 + b3).reshape(B, 1, -1)

Distribution: pure data parallel — batch 2048 sharded 256/core across 8
NeuronCores, weights replicated. No collectives; host concatenates shards.

Host-side packing: x is cast to bf16 (the on-device pipeline consumes bf16
anyway — same numerics as the previous in-flight DMA cast, half the HBM
read bytes) and deinterleaved so each conv tap k is a contiguous 512-wide
plane per channel (x_re[b,c,k,f] = x[b,c,4f+k]); W2 is packed bf16 in
2-k-tile row pairs (4KB DMA rows); conv bias b1 is folded through W2 into
b2'.

Per-core device pipeline, streaming over 12 channels (48 k-tiles of the
6144-dim contraction; one k-tile = one (channel, 128-feature block)):
  1. x streams on the sync-engine HWDGE queue ([128 b, 2048] bf16 channel
     tiles, 4KB rows); W2 streams on the scalar-engine HWDGE queue (bf16
     k-pair tiles, 4KB rows). Two HW queues so neither blocks the other.
  2. DVE computes the conv as a 7-op tree over the 4 tap planes (4x
     tensor_scalar_mul with the tap weight as a per-partition f32 scalar
     AP + 3x tensor_tensor add, all packed bf16) -> conv [128 b, 512 f].
  3. TensorE transposes conv 128-f slices to [f, b] (2 per k-tile, PSUM),
     ScalarE copies PSUM -> SBUF -> flatT k-tile [128 d, 256 b].
  4. TensorE accumulates flatT against W2 k-tiles into a persistent PSUM
     accumulator [256 b, 1024 h] (4 banks, one accumulation group each —
     start=True clears has_written at bank granularity, so groups must
     not share banks).
  5. Epilogue: DVE/ACT copy raw fp32 h to SBUF, TensorE transposes to
     [h, b], ACT applies relu(h + b2') via per-partition bias, casting to
     bf16.
  6. TensorE MLP2: hT against W3 -> [256 b, 256 o], DVE adds b3, DMA out.

All constants are DMA'd at kernel start so the epilogue never waits.
"""

from contextlib import ExitStack

import ml_dtypes
import numpy as np

import concourse.bacc as bacc
import concourse.tile as tile
from concourse import mybir
from concourse.bass_utils import run_bass_kernel_spmd

BF16 = ml_dtypes.bfloat16

B, C, L = 2048, 12, 2048
STEP = 4
F = L // STEP               # 512 features per channel
DIN = C * F                 # 6144
HID = 1024
OUT = 256
NCORES = 8
BL = B // NCORES            # 256 batch rows per core
KT = DIN // 128             # 48 k-tiles


def _emit(nc, tc, ctx, x_ap, w2_ap, w3_ap, w1rep_ap, bias2_ap, b3rep_ap, ident_ap, identf_ap, out_ap):
    bf16, f32 = mybir.dt.bfloat16, mybir.dt.float32
    add, mult = mybir.AluOpType.add, mybir.AluOpType.mult

    const = ctx.enter_context(tc.tile_pool(name="const", bufs=1))
    w1rep_s = const.tile([128, 4 * C], f32, name="w1rep_s")
    nc.sync.dma_start(w1rep_s[:], w1rep_ap[:])
    ident_s = const.tile([128, 128], bf16, name="ident_s")
    nc.sync.dma_start(ident_s[:], ident_ap[:])
    # Epilogue constants ride the scalar HW queue ahead of W2: resident long
    # before the epilogue without delaying the first x tiles on sync.
    ident_f32_s = const.tile([128, 128], f32, name="ident_f32_s")
    nc.scalar.dma_start(ident_f32_s[:], identf_ap[:])
    bias2_s = const.tile([128, 8], f32, name="bias2_s")
    nc.scalar.dma_start(bias2_s[:], bias2_ap[:])
    b3rep_s = const.tile([128, OUT], f32, name="b3rep_s")
    nc.scalar.dma_start(b3rep_s[:], b3rep_ap[:])
    w3_s = const.tile([128, 8 * OUT], bf16, name="w3_s")
    nc.scalar.dma_start(
        w3_s.rearrange("p (k n) -> p k n", k=8),
        w3_ap.rearrange("(k p) n -> p k n", p=128),
    )

    # Persistent MLP1 accumulator in [batch, hidden] orientation: 4 PSUM
    # banks [128 b, 512 h], indexed [2*bt + hh]. One accumulation group per
    # bank — PE's start=True clears has_written at bank granularity, so two
    # interleaved groups must never share a bank.
    ps1_pool = ctx.enter_context(tc.tile_pool(name="ps1", bufs=1, space="PSUM"))
    ps1 = [ps1_pool.tile([128, 512], f32, name=f"ps1_{i}") for i in range(4)]

    relu_pool = ctx.enter_context(tc.tile_pool(name="hts", bufs=1))
    outs_pool = ctx.enter_context(tc.tile_pool(name="outs", bufs=2))

    # W2 is fully SBUF-resident (24 k-pair tiles, 96KB/partition, never
    # recycled -> no load ever blocks an engine on a pool semaphore). The
    # first 8 pairs ride the two HW queues interleaved with x (front-loaded
    # for the early channels); the remaining 16 stream on the gpsimd SWDGE
    # queue, whose descriptor-gens are spread across channels 0-7.
    w2all_pool = ctx.enter_context(tc.tile_pool(name="w2all", bufs=1))
    w2all = [
        w2all_pool.tile([128, 2 * HID], bf16, name=f"w2_{g}") for g in range(2 * C)
    ]

    with ExitStack() as kctx:
        xnat = kctx.enter_context(tc.tile_pool(name="xnat", bufs=8))
        cint = kctx.enter_context(tc.tile_pool(name="cint", bufs=3))
        cvs = kctx.enter_context(tc.tile_pool(name="cvs", bufs=4))
        ftp = kctx.enter_context(tc.tile_pool(name="ftp", bufs=2, space="PSUM"))
        fts = kctx.enter_context(tc.tile_pool(name="fts", bufs=6))

        for c in range(C):
            # One [128 b, 2048] bf16 tile per batch-half holds all 4 tap
            # planes of channel c (4KB rows). x alternates between the two
            # HW queues so both sets of DMA engines stay fed.
            xq = nc.sync if c % 2 == 0 else nc.scalar
            xt = [None, None]
            for bh in range(2):
                xa = xnat.tile([128, 2048], bf16, name="xa")
                xq.dma_start(xa[:], x_ap[128 * bh : 128 * (bh + 1), c, :])
                xt[bh] = xa

            if c < 4:
                # W2 pairs g=0..7 on the HW queues, behind this channel's x.
                nc.sync.dma_start(
                    w2all[2 * c][:], w2_ap[256 * c : 256 * c + 128, :]
                )
                nc.scalar.dma_start(
                    w2all[2 * c + 1][:], w2_ap[256 * c + 128 : 256 * (c + 1), :]
                )
            if c < 8:
                # W2 pairs g=8..23 on SWDGE, 2 descriptor-gens per channel so
                # they never head-of-line block gpsimd's s23 adds.
                for g in (8 + 2 * c, 9 + 2 * c):
                    nc.gpsimd.dma_start(
                        w2all[g][:], w2_ap[128 * g : 128 * (g + 1), :]
                    )

            w2pair = [w2all[2 * c], w2all[2 * c + 1]]

            # Conv: 4 packed tensor_scalar muls + a 3-add tree. The s23 add
            # runs on gpsimd (otherwise idle); everything else on DVE. Only
            # plain tensor_scalar/tensor_tensor get the 2x DVE fast mode.
            ftx = [None, None]
            for bh in range(2):
                xs = xt[bh]
                m = []
                for k4 in range(4):
                    mk = cint.tile([128, 512], bf16, name=f"m{k4}")
                    nc.vector.tensor_scalar_mul(
                        mk[:],
                        xs[:, 512 * k4 : 512 * (k4 + 1)],
                        w1rep_s[:, 4 * c + k4 : 4 * c + k4 + 1],
                    )
                    m.append(mk)
                s01 = cint.tile([128, 512], bf16, name="s01")
                nc.vector.tensor_tensor(s01[:], m[0][:], m[1][:], add)
                s23 = cint.tile([128, 512], bf16, name="s23")
                nc.gpsimd.tensor_tensor(s23[:], m[2][:], m[3][:], add)
                cv = cvs.tile([128, 512], bf16, name="cv")
                nc.vector.tensor_tensor(cv[:], s01[:], s23[:], add)
                ftx[bh] = cv

            for j2 in range(4):
                k = 4 * c + j2
                w2t = w2pair[j2 // 2][:, HID * (j2 % 2) : HID * (j2 % 2 + 1)]

                # flatT k-tile: transpose conv f-slices [128 b, 128 f] ->
                # [128 f, 128 b], col-blocked [b0 | b1] in one PSUM tile.
                ftpt = ftp.tile([128, 256], bf16, name="ftpt")
                for bh in range(2):
                    nc.tensor.transpose(
                        ftpt[:, 128 * bh : 128 * (bh + 1)],
                        ftx[bh][:, 128 * j2 : 128 * (j2 + 1)],
                        ident_s[:],
                    )
                ft = fts.tile([128, 256], bf16, name="ft")
                nc.scalar.copy(ft[:], ftpt[:])

                for bt in range(2):
                    for hh in range(2):
                        nc.tensor.matmul(
                            ps1[2 * bt + hh],
                            ft[:, 128 * bt : 128 * bt + 128],
                            w2t[:, 512 * hh : 512 * (hh + 1)],
                            start=(k == 0),
                            stop=(k == KT - 1),
                        )

    # Epilogue: copy raw fp32 h [b, 1024] to SBUF, PE-transpose to [h, b],
    # then ACT relu(h + b2') with per-partition bias, casting to bf16.
    hraw = []
    for bt in range(2):
        hr = relu_pool.tile([128, HID], f32, name=f"hraw{bt}")
        for hh in range(2):
            src = ps1[2 * bt + hh][:]
            dst = hr[:, 512 * hh : 512 * (hh + 1)]
            if bt == 0:
                nc.vector.tensor_copy(dst, src)
            else:
                nc.scalar.copy(dst, src)
        hraw.append(hr)

    hts = []
    htp_pool = ctx.enter_context(tc.tile_pool(name="htp", bufs=2, space="PSUM"))
    for p in range(4):  # k2-pairs
        tileT = htp_pool.tile([128, 512], f32, name="tileT")
        for q in range(2):  # k2 = 2p + q
            k2 = 2 * p + q
            for bt in range(2):
                nc.tensor.transpose(
                    tileT[:, 256 * q + 128 * bt : 256 * q + 128 * bt + 128],
                    hraw[bt][:, 128 * k2 : 128 * (k2 + 1)],
                    ident_f32_s[:],
                )
        for q in range(2):
            k2 = 2 * p + q
            ht = relu_pool.tile([128, 256], bf16, name=f"ht{k2}")
            nc.scalar.activation(
                ht[:],
                tileT[:, 256 * q : 256 * q + 256],
                mybir.ActivationFunctionType.Relu,
                bias=bias2_s[:, k2 : k2 + 1],
                scale=1.0,
            )
            hts.append(ht)

    # MLP2: out[b, o] per 128-row batch tile, then + b3 and DMA out.
    ps2_pool = ctx.enter_context(tc.tile_pool(name="ps2", bufs=2, space="PSUM"))
    for bt in range(2):
        p2 = ps2_pool.tile([128, OUT], f32, name="p2")
        for k2 in range(8):
            nc.tensor.matmul(
                p2[:],
                hts[k2][:, 128 * bt : 128 * bt + 128],
                w3_s[:, 256 * k2 : 256 * k2 + 256],
                start=(k2 == 0),
                stop=(k2 == 7),
            )
        ob = outs_pool.tile([128, OUT], f32, name="ob")
        nc.vector.tensor_add(ob[:], p2[:], b3rep_s[:])
        nc.sync.dma_start(out_ap[128 * bt : 128 * (bt + 1), :], ob[:])


_BUILT = {}


def _build():
    if "nc" in _BUILT:
        return _BUILT["nc"]
    nc = bacc.Bacc("TRN2", target_bir_lowering=False, debug=False)
    bf16, f32 = mybir.dt.bfloat16, mybir.dt.float32
    x_t = nc.dram_tensor("x", [BL, C, L], bf16, kind="ExternalInput")
    w2_t = nc.dram_tensor("w2", [DIN // 2, 2 * HID], bf16, kind="ExternalInput")
    w3_t = nc.dram_tensor("w3", [HID, OUT], bf16, kind="ExternalInput")
    w1rep_t = nc.dram_tensor("w1rep", [128, 4 * C], f32, kind="ExternalInput")
    bias2_t = nc.dram_tensor("bias2", [128, 8], f32, kind="ExternalInput")
    b3rep_t = nc.dram_tensor("b3rep", [128, OUT], f32, kind="ExternalInput")
    ident_t = nc.dram_tensor("ident", [128, 128], bf16, kind="ExternalInput")
    identf_t = nc.dram_tensor("identf", [128, 128], f32, kind="ExternalInput")
    out_t = nc.dram_tensor("out", [BL, OUT], f32, kind="ExternalOutput")
    with tile.TileContext(nc) as tc, ExitStack() as ctx:
        _emit(
            nc,
            tc,
            ctx,
            x_t.ap(),
            w2_t.ap(),
            w3_t.ap(),
            w1rep_t.ap(),
            bias2_t.ap(),
            b3rep_t.ap(),
            ident_t.ap(),
            identf_t.ap(),
            out_t.ap(),
        )
    nc.compile()
    _BUILT["nc"] = nc
    return nc


def _pack_weights(W1, b1, W2, b2, W3, b3):
    W1 = np.asarray(W1, np.float32)
    b1 = np.asarray(b1, np.float32)
    W2 = np.asarray(W2, np.float32)
    b2 = np.asarray(b2, np.float32)
    W3 = np.asarray(W3, np.float32)
    b3 = np.asarray(b3, np.float32)

    # Per-partition tap scalars: w1rep[p, 4c + k] = W1[c, k].
    w1rep = np.ascontiguousarray(
        np.broadcast_to(W1.reshape(1, 4 * C), (128, 4 * C))
    ).astype(np.float32)

    # Fold conv bias through W2: b2' = b2 + b1 @ sum_f W2[c*F+f, :].
    b2p = b2 + b1 @ W2.reshape(C, F, HID).sum(axis=1)
    bias2 = np.ascontiguousarray(b2p.reshape(8, 128).T).astype(np.float32)

    b3rep = np.ascontiguousarray(np.broadcast_to(b3, (128, OUT))).astype(np.float32)
    ident = np.eye(128, dtype=BF16)
    # Pack W2 so each DMA partition-row carries a contiguous 4KB k-pair:
    # packed[g*128 + p, :] = [W2[256g + p, :] | W2[256g + 128 + p, :]].
    w2b = W2.astype(BF16)
    w2packed = np.ascontiguousarray(
        w2b.reshape(DIN // 256, 2, 128, HID).swapaxes(1, 2).reshape(DIN // 2, 2 * HID)
    )
    return dict(
        w2=w2packed,
        w3=np.ascontiguousarray(W3.astype(BF16)),
        w1rep=w1rep,
        bias2=bias2,
        b3rep=b3rep,
        ident=ident,
        identf=np.eye(128, dtype=np.float32),
    )


def kernel(x, W1, b1, W2, b2, W3, b3, _trace=False):
    x = np.asarray(x, np.float32)
    # Deinterleave conv taps (x_re[b, c, k*F + f] = x[b, c, 4f + k]) and cast
    # to bf16 — the device pipeline consumes x as bf16 either way.
    x = np.ascontiguousarray(
        x.reshape(B, C, F, STEP).transpose(0, 1, 3, 2).reshape(B, C, L).astype(BF16)
    )
    nc = _build()
    shared = _pack_weights(W1, b1, W2, b2, W3, b3)
    in_maps = [dict(shared, x=x[i * BL : (i + 1) * BL]) for i in range(NCORES)]
    res = run_bass_kernel_spmd(nc, in_maps, list(range(NCORES)), trace=_trace)
    out = np.concatenate([res.results[i]["out"] for i in range(NCORES)], axis=0)
    out = out.reshape(B, 1, OUT)
    if _trace:
        kernel.last_results = res
    return out


# revision 9
# speedup vs baseline: 1.1425x; 1.1425x over previous
"""Trainium2 Bass kernel for nn_CNNMode_Kernal_2 (dense_cnn).

Reference computation (all fp32):
    xp = x.reshape(B, C, L//4, 4)
    conv[b,c,f] = sum_k xp[b,c,f,k] * W1[c,k] + b1[c]          # per-channel Conv1d(1,1,4,4)
    flat = conv.reshape(B, C*F)                                 # channel-major
    h = relu(flat @ W2 + b2)
    out = (h @ W3 + b3).reshape(B, 1, -1)

Distribution: pure data parallel — batch 2048 sharded 256/core across 8
NeuronCores, weights replicated. No collectives; host concatenates shards.

Host-side packing: x is cast to bf16 (the on-device pipeline consumes bf16
anyway — same numerics as the previous in-flight DMA cast, half the HBM
read bytes) and deinterleaved so each conv tap k is a contiguous 512-wide
plane per channel (x_re[b,c,k,f] = x[b,c,4f+k]); W2 is packed bf16 in
2-k-tile row pairs (4KB DMA rows); conv bias b1 is folded through W2 into
b2'.

Per-core device pipeline, streaming over 12 channels (48 k-tiles of the
6144-dim contraction; one k-tile = one (channel, 128-feature block)):
  1. x streams on the sync-engine HWDGE queue ([128 b, 2048] bf16 channel
     tiles, 4KB rows); W2 streams on the scalar-engine HWDGE queue (bf16
     k-pair tiles, 4KB rows). Two HW queues so neither blocks the other.
  2. DVE computes the conv as a 7-op tree over the 4 tap planes (4x
     tensor_scalar_mul with the tap weight as a per-partition f32 scalar
     AP + 3x tensor_tensor add, all packed bf16) -> conv [128 b, 512 f].
  3. TensorE transposes conv 128-f slices to [f, b] (2 per k-tile, PSUM),
     ScalarE copies PSUM -> SBUF -> flatT k-tile [128 d, 256 b].
  4. TensorE accumulates flatT against W2 k-tiles into a persistent PSUM
     accumulator [256 b, 1024 h] (4 banks, one accumulation group each —
     start=True clears has_written at bank granularity, so groups must
     not share banks).
  5. Epilogue: DVE/ACT copy raw fp32 h to SBUF, TensorE transposes to
     [h, b], ACT applies relu(h + b2') via per-partition bias, casting to
     bf16.
  6. TensorE MLP2: hT against W3 -> [256 b, 256 o], DVE adds b3, DMA out.

All constants are DMA'd at kernel start so the epilogue never waits.
"""

from contextlib import ExitStack

import ml_dtypes
import numpy as np

import concourse.bacc as bacc
import concourse.tile as tile
from concourse import mybir
from concourse.bass_utils import run_bass_kernel_spmd

BF16 = ml_dtypes.bfloat16

B, C, L = 2048, 12, 2048
STEP = 4
F = L // STEP               # 512 features per channel
DIN = C * F                 # 6144
HID = 1024
OUT = 256
NCORES = 8
BL = B // NCORES            # 256 batch rows per core
KT = DIN // 128             # 48 k-tiles


def _emit(nc, tc, ctx, x_ap, w2_ap, w3_ap, w1rep_ap, bias2_ap, b3rep_ap, ident_ap, identf_ap, out_ap):
    bf16, f32 = mybir.dt.bfloat16, mybir.dt.float32
    add, mult = mybir.AluOpType.add, mybir.AluOpType.mult

    const = ctx.enter_context(tc.tile_pool(name="const", bufs=1))
    w1rep_s = const.tile([128, 4 * C], f32, name="w1rep_s")
    nc.sync.dma_start(w1rep_s[:], w1rep_ap[:])
    ident_s = const.tile([128, 128], bf16, name="ident_s")
    nc.sync.dma_start(ident_s[:], ident_ap[:])
    # Epilogue constants ride the scalar HW queue ahead of W2: resident long
    # before the epilogue without delaying the first x tiles on sync.
    ident_f32_s = const.tile([128, 128], f32, name="ident_f32_s")
    nc.scalar.dma_start(ident_f32_s[:], identf_ap[:])
    bias2_s = const.tile([128, 8], f32, name="bias2_s")
    nc.scalar.dma_start(bias2_s[:], bias2_ap[:])
    b3rep_s = const.tile([128, OUT], f32, name="b3rep_s")
    nc.scalar.dma_start(b3rep_s[:], b3rep_ap[:])
    w3_s = const.tile([128, 8 * OUT], bf16, name="w3_s")
    nc.scalar.dma_start(
        w3_s.rearrange("p (k n) -> p k n", k=8),
        w3_ap.rearrange("(k p) n -> p k n", p=128),
    )

    # Persistent MLP1 accumulator in [batch, hidden] orientation: 4 PSUM
    # banks [128 b, 512 h], indexed [2*bt + hh]. One accumulation group per
    # bank — PE's start=True clears has_written at bank granularity, so two
    # interleaved groups must never share a bank.
    ps1_pool = ctx.enter_context(tc.tile_pool(name="ps1", bufs=1, space="PSUM"))
    ps1 = [ps1_pool.tile([128, 512], f32, name=f"ps1_{i}") for i in range(4)]

    relu_pool = ctx.enter_context(tc.tile_pool(name="hts", bufs=1))
    outs_pool = ctx.enter_context(tc.tile_pool(name="outs", bufs=2))

    with ExitStack() as kctx:
        xnat = kctx.enter_context(tc.tile_pool(name="xnat", bufs=8))
        w2p = kctx.enter_context(tc.tile_pool(name="w2p", bufs=8))
        cint = kctx.enter_context(tc.tile_pool(name="cint", bufs=3))
        cvs = kctx.enter_context(tc.tile_pool(name="cvs", bufs=4))
        ftp = kctx.enter_context(tc.tile_pool(name="ftp", bufs=2, space="PSUM"))
        fts = kctx.enter_context(tc.tile_pool(name="fts", bufs=6))

        for c in range(C):
            # One [128 b, 2048] bf16 tile per batch-half holds all 4 tap
            # planes of channel c (4KB rows).
            xt = [None, None]
            for bh in range(2):
                xa = xnat.tile([128, 2048], bf16, name="xa")
                nc.sync.dma_start(xa[:], x_ap[128 * bh : 128 * (bh + 1), c, :])
                xt[bh] = xa

            w2pair = [None, None]
            for half in range(2):
                # One 4KB-per-row DMA covers a k-pair (host-packed rows).
                g = 2 * c + half
                w2t2 = w2p.tile([128, 2 * HID], bf16, name="w2t")
                nc.scalar.dma_start(w2t2[:], w2_ap[128 * g : 128 * (g + 1), :])
                w2pair[half] = w2t2

            # Conv: 4 packed tensor_scalar muls + a 3-add tree. The s23 add
            # runs on gpsimd (otherwise idle); everything else on DVE. Only
            # plain tensor_scalar/tensor_tensor get the 2x DVE fast mode.
            ftx = [None, None]
            for bh in range(2):
                xs = xt[bh]
                m = []
                for k4 in range(4):
                    mk = cint.tile([128, 512], bf16, name=f"m{k4}")
                    nc.vector.tensor_scalar_mul(
                        mk[:],
                        xs[:, 512 * k4 : 512 * (k4 + 1)],
                        w1rep_s[:, 4 * c + k4 : 4 * c + k4 + 1],
                    )
                    m.append(mk)
                s01 = cint.tile([128, 512], bf16, name="s01")
                nc.vector.tensor_tensor(s01[:], m[0][:], m[1][:], add)
                s23 = cint.tile([128, 512], bf16, name="s23")
                nc.vector.tensor_tensor(s23[:], m[2][:], m[3][:], add)
                cv = cvs.tile([128, 512], bf16, name="cv")
                nc.vector.tensor_tensor(cv[:], s01[:], s23[:], add)
                ftx[bh] = cv

            for j2 in range(4):
                k = 4 * c + j2
                w2t = w2pair[j2 // 2][:, HID * (j2 % 2) : HID * (j2 % 2 + 1)]

                # flatT k-tile: transpose conv f-slices [128 b, 128 f] ->
                # [128 f, 128 b], col-blocked [b0 | b1] in one PSUM tile.
                ftpt = ftp.tile([128, 256], bf16, name="ftpt")
                for bh in range(2):
                    nc.tensor.transpose(
                        ftpt[:, 128 * bh : 128 * (bh + 1)],
                        ftx[bh][:, 128 * j2 : 128 * (j2 + 1)],
                        ident_s[:],
                    )
                ft = fts.tile([128, 256], bf16, name="ft")
                nc.scalar.copy(ft[:], ftpt[:])

                for bt in range(2):
                    for hh in range(2):
                        nc.tensor.matmul(
                            ps1[2 * bt + hh],
                            ft[:, 128 * bt : 128 * bt + 128],
                            w2t[:, 512 * hh : 512 * (hh + 1)],
                            start=(k == 0),
                            stop=(k == KT - 1),
                        )

    # Epilogue: copy raw fp32 h [b, 1024] to SBUF, PE-transpose to [h, b],
    # then ACT relu(h + b2') with per-partition bias, casting to bf16.
    hraw = []
    for bt in range(2):
        hr = relu_pool.tile([128, HID], f32, name=f"hraw{bt}")
        for hh in range(2):
            src = ps1[2 * bt + hh][:]
            dst = hr[:, 512 * hh : 512 * (hh + 1)]
            if bt == 0:
                nc.vector.tensor_copy(dst, src)
            else:
                nc.scalar.copy(dst, src)
        hraw.append(hr)

    hts = []
    htp_pool = ctx.enter_context(tc.tile_pool(name="htp", bufs=2, space="PSUM"))
    for p in range(4):  # k2-pairs
        tileT = htp_pool.tile([128, 512], f32, name="tileT")
        for q in range(2):  # k2 = 2p + q
            k2 = 2 * p + q
            for bt in range(2):
                nc.tensor.transpose(
                    tileT[:, 256 * q + 128 * bt : 256 * q + 128 * bt + 128],
                    hraw[bt][:, 128 * k2 : 128 * (k2 + 1)],
                    ident_f32_s[:],
                )
        for q in range(2):
            k2 = 2 * p + q
            ht = relu_pool.tile([128, 256], bf16, name=f"ht{k2}")
            nc.scalar.activation(
                ht[:],
                tileT[:, 256 * q : 256 * q + 256],
                mybir.ActivationFunctionType.Relu,
                bias=bias2_s[:, k2 : k2 + 1],
                scale=1.0,
            )
            hts.append(ht)

    # MLP2: out[b, o] per 128-row batch tile, then + b3 and DMA out.
    ps2_pool = ctx.enter_context(tc.tile_pool(name="ps2", bufs=2, space="PSUM"))
    for bt in range(2):
        p2 = ps2_pool.tile([128, OUT], f32, name="p2")
        for k2 in range(8):
            nc.tensor.matmul(
                p2[:],
                hts[k2][:, 128 * bt : 128 * bt + 128],
                w3_s[:, 256 * k2 : 256 * k2 + 256],
                start=(k2 == 0),
                stop=(k2 == 7),
            )
        ob = outs_pool.tile([128, OUT], f32, name="ob")
        nc.vector.tensor_add(ob[:], p2[:], b3rep_s[:])
        nc.sync.dma_start(out_ap[128 * bt : 128 * (bt + 1), :], ob[:])


_BUILT = {}


def _build():
    if "nc" in _BUILT:
        return _BUILT["nc"]
    nc = bacc.Bacc("TRN2", target_bir_lowering=False, debug=False)
    bf16, f32 = mybir.dt.bfloat16, mybir.dt.float32
    x_t = nc.dram_tensor("x", [BL, C, L], bf16, kind="ExternalInput")
    w2_t = nc.dram_tensor("w2", [DIN // 2, 2 * HID], bf16, kind="ExternalInput")
    w3_t = nc.dram_tensor("w3", [HID, OUT], bf16, kind="ExternalInput")
    w1rep_t = nc.dram_tensor("w1rep", [128, 4 * C], f32, kind="ExternalInput")
    bias2_t = nc.dram_tensor("bias2", [128, 8], f32, kind="ExternalInput")
    b3rep_t = nc.dram_tensor("b3rep", [128, OUT], f32, kind="ExternalInput")
    ident_t = nc.dram_tensor("ident", [128, 128], bf16, kind="ExternalInput")
    identf_t = nc.dram_tensor("identf", [128, 128], f32, kind="ExternalInput")
    out_t = nc.dram_tensor("out", [BL, OUT], f32, kind="ExternalOutput")
    with tile.TileContext(nc) as tc, ExitStack() as ctx:
        _emit(
            nc,
            tc,
            ctx,
            x_t.ap(),
            w2_t.ap(),
            w3_t.ap(),
            w1rep_t.ap(),
            bias2_t.ap(),
            b3rep_t.ap(),
            ident_t.ap(),
            identf_t.ap(),
            out_t.ap(),
        )
    nc.compile()
    _BUILT["nc"] = nc
    return nc


def _pack_weights(W1, b1, W2, b2, W3, b3):
    W1 = np.asarray(W1, np.float32)
    b1 = np.asarray(b1, np.float32)
    W2 = np.asarray(W2, np.float32)
    b2 = np.asarray(b2, np.float32)
    W3 = np.asarray(W3, np.float32)
    b3 = np.asarray(b3, np.float32)

    # Per-partition tap scalars: w1rep[p, 4c + k] = W1[c, k].
    w1rep = np.ascontiguousarray(
        np.broadcast_to(W1.reshape(1, 4 * C), (128, 4 * C))
    ).astype(np.float32)

    # Fold conv bias through W2: b2' = b2 + b1 @ sum_f W2[c*F+f, :].
    b2p = b2 + b1 @ W2.reshape(C, F, HID).sum(axis=1)
    bias2 = np.ascontiguousarray(b2p.reshape(8, 128).T).astype(np.float32)

    b3rep = np.ascontiguousarray(np.broadcast_to(b3, (128, OUT))).astype(np.float32)
    ident = np.eye(128, dtype=BF16)
    # Pack W2 so each DMA partition-row carries a contiguous 4KB k-pair:
    # packed[g*128 + p, :] = [W2[256g + p, :] | W2[256g + 128 + p, :]].
    w2b = W2.astype(BF16)
    w2packed = np.ascontiguousarray(
        w2b.reshape(DIN // 256, 2, 128, HID).swapaxes(1, 2).reshape(DIN // 2, 2 * HID)
    )
    return dict(
        w2=w2packed,
        w3=np.ascontiguousarray(W3.astype(BF16)),
        w1rep=w1rep,
        bias2=bias2,
        b3rep=b3rep,
        ident=ident,
        identf=np.eye(128, dtype=np.float32),
    )


def kernel(x, W1, b1, W2, b2, W3, b3, _trace=False):
    x = np.asarray(x, np.float32)
    # Deinterleave conv taps (x_re[b, c, k*F + f] = x[b, c, 4f + k]) and cast
    # to bf16 — the device pipeline consumes x as bf16 either way.
    x = np.ascontiguousarray(
        x.reshape(B, C, F, STEP).transpose(0, 1, 3, 2).reshape(B, C, L).astype(BF16)
    )
    nc = _build()
    shared = _pack_weights(W1, b1, W2, b2, W3, b3)
    in_maps = [dict(shared, x=x[i * BL : (i + 1) * BL]) for i in range(NCORES)]
    res = run_bass_kernel_spmd(nc, in_maps, list(range(NCORES)), trace=_trace)
    out = np.concatenate([res.results[i]["out"] for i in range(NCORES)], axis=0)
    out = out.reshape(B, 1, OUT)
    if _trace:
        kernel.last_results = res
    return out


# revision 10
# speedup vs baseline: 1.1469x; 1.0038x over previous
"""Trainium2 Bass kernel for nn_CNNMode_Kernal_2 (dense_cnn).

Reference computation (all fp32):
    xp = x.reshape(B, C, L//4, 4)
    conv[b,c,f] = sum_k xp[b,c,f,k] * W1[c,k] + b1[c]          # per-channel Conv1d(1,1,4,4)
    flat = conv.reshape(B, C*F)                                 # channel-major
    h = relu(flat @ W2 + b2)
    out = (h @ W3 + b3).reshape(B, 1, -1)

Distribution: pure data parallel — batch 2048 sharded 256/core across 8
NeuronCores, weights replicated. No collectives; host concatenates shards.

Host-side packing: x is cast to bf16 (the on-device pipeline consumes bf16
anyway — same numerics as the previous in-flight DMA cast, half the HBM
read bytes) and deinterleaved so each conv tap k is a contiguous 512-wide
plane per channel (x_re[b,c,k,f] = x[b,c,4f+k]); W2 is packed bf16 in
2-k-tile row pairs (4KB DMA rows); conv bias b1 is folded through W2 into
b2'.

Per-core device pipeline, streaming over 12 channels (48 k-tiles of the
6144-dim contraction; one k-tile = one (channel, 128-feature block)):
  1. x streams on the sync-engine HWDGE queue ([128 b, 2048] bf16 channel
     tiles, 4KB rows); W2 streams on the scalar-engine HWDGE queue (bf16
     k-pair tiles, 4KB rows). Two HW queues so neither blocks the other.
  2. DVE computes the conv as a 7-op tree over the 4 tap planes (4x
     tensor_scalar_mul with the tap weight as a per-partition f32 scalar
     AP + 3x tensor_tensor add, all packed bf16) -> conv [128 b, 512 f].
  3. TensorE transposes conv 128-f slices to [f, b] (2 per k-tile, PSUM),
     ScalarE copies PSUM -> SBUF -> flatT k-tile [128 d, 256 b].
  4. TensorE accumulates flatT against W2 k-tiles into a persistent PSUM
     accumulator [256 b, 1024 h] (4 banks, one accumulation group each —
     start=True clears has_written at bank granularity, so groups must
     not share banks).
  5. Epilogue: DVE/ACT copy raw fp32 h to SBUF, TensorE transposes to
     [h, b], ACT applies relu(h + b2') via per-partition bias, casting to
     bf16.
  6. TensorE MLP2: hT against W3 -> [256 b, 256 o], DVE adds b3, DMA out.

All constants are DMA'd at kernel start so the epilogue never waits.
"""

from contextlib import ExitStack

import ml_dtypes
import numpy as np

import concourse.bacc as bacc
import concourse.tile as tile
from concourse import mybir
from concourse.bass_utils import run_bass_kernel_spmd

BF16 = ml_dtypes.bfloat16

B, C, L = 2048, 12, 2048
STEP = 4
F = L // STEP               # 512 features per channel
DIN = C * F                 # 6144
HID = 1024
OUT = 256
NCORES = 8
BL = B // NCORES            # 256 batch rows per core
KT = DIN // 128             # 48 k-tiles


def _emit(nc, tc, ctx, x_ap, w2_ap, w3_ap, w1rep_ap, bias2_ap, b3rep_ap, ident_ap, identf_ap, out_ap):
    bf16, f32 = mybir.dt.bfloat16, mybir.dt.float32
    add, mult = mybir.AluOpType.add, mybir.AluOpType.mult

    const = ctx.enter_context(tc.tile_pool(name="const", bufs=1))
    w1rep_s = const.tile([128, 4 * C], f32, name="w1rep_s")
    nc.sync.dma_start(w1rep_s[:], w1rep_ap[:])
    ident_s = const.tile([128, 128], bf16, name="ident_s")
    nc.sync.dma_start(ident_s[:], ident_ap[:])
    # Epilogue constants ride the scalar HW queue ahead of W2: resident long
    # before the epilogue without delaying the first x tiles on sync.
    ident_f32_s = const.tile([128, 128], f32, name="ident_f32_s")
    nc.scalar.dma_start(ident_f32_s[:], identf_ap[:])
    bias2_s = const.tile([128, 8], f32, name="bias2_s")
    nc.scalar.dma_start(bias2_s[:], bias2_ap[:])
    b3rep_s = const.tile([128, OUT], f32, name="b3rep_s")
    nc.scalar.dma_start(b3rep_s[:], b3rep_ap[:])
    w3_s = const.tile([128, 8 * OUT], bf16, name="w3_s")
    nc.scalar.dma_start(
        w3_s.rearrange("p (k n) -> p k n", k=8),
        w3_ap.rearrange("(k p) n -> p k n", p=128),
    )

    # Persistent MLP1 accumulator in [batch, hidden] orientation: 4 PSUM
    # banks [128 b, 512 h], indexed [2*bt + hh]. One accumulation group per
    # bank — PE's start=True clears has_written at bank granularity, so two
    # interleaved groups must never share a bank.
    ps1_pool = ctx.enter_context(tc.tile_pool(name="ps1", bufs=1, space="PSUM"))
    ps1 = [ps1_pool.tile([128, 512], f32, name=f"ps1_{i}") for i in range(4)]

    relu_pool = ctx.enter_context(tc.tile_pool(name="hts", bufs=1))
    outs_pool = ctx.enter_context(tc.tile_pool(name="outs", bufs=2))

    with ExitStack() as kctx:
        xnat = kctx.enter_context(tc.tile_pool(name="xnat", bufs=10))
        w2p = kctx.enter_context(tc.tile_pool(name="w2p", bufs=8))
        cint = kctx.enter_context(tc.tile_pool(name="cint", bufs=4))
        cvs = kctx.enter_context(tc.tile_pool(name="cvs", bufs=4))
        ftp = kctx.enter_context(tc.tile_pool(name="ftp", bufs=2, space="PSUM"))
        fts = kctx.enter_context(tc.tile_pool(name="fts", bufs=8))

        for c in range(C):
            # One [128 b, 2048] bf16 tile per batch-half holds all 4 tap
            # planes of channel c (4KB rows).
            xt = [None, None]
            for bh in range(2):
                xa = xnat.tile([128, 2048], bf16, name="xa")
                nc.sync.dma_start(xa[:], x_ap[128 * bh : 128 * (bh + 1), c, :])
                xt[bh] = xa

            w2pair = [None, None]
            for half in range(2):
                # One 4KB-per-row DMA covers a k-pair (host-packed rows).
                g = 2 * c + half
                w2t2 = w2p.tile([128, 2 * HID], bf16, name="w2t")
                nc.scalar.dma_start(w2t2[:], w2_ap[128 * g : 128 * (g + 1), :])
                w2pair[half] = w2t2

            # Conv: 4 packed tensor_scalar muls + a 3-add tree. The s23 add
            # runs on gpsimd (otherwise idle); everything else on DVE. Only
            # plain tensor_scalar/tensor_tensor get the 2x DVE fast mode.
            ftx = [None, None]
            for bh in range(2):
                xs = xt[bh]
                m = []
                for k4 in range(4):
                    mk = cint.tile([128, 512], bf16, name=f"m{k4}")
                    nc.vector.tensor_scalar_mul(
                        mk[:],
                        xs[:, 512 * k4 : 512 * (k4 + 1)],
                        w1rep_s[:, 4 * c + k4 : 4 * c + k4 + 1],
                    )
                    m.append(mk)
                s01 = cint.tile([128, 512], bf16, name="s01")
                nc.vector.tensor_tensor(s01[:], m[0][:], m[1][:], add)
                s23 = cint.tile([128, 512], bf16, name="s23")
                nc.gpsimd.tensor_tensor(s23[:], m[2][:], m[3][:], add)
                cv = cvs.tile([128, 512], bf16, name="cv")
                nc.vector.tensor_tensor(cv[:], s01[:], s23[:], add)
                ftx[bh] = cv

            for j2 in range(4):
                k = 4 * c + j2
                w2t = w2pair[j2 // 2][:, HID * (j2 % 2) : HID * (j2 % 2 + 1)]

                # flatT k-tile: transpose conv f-slices [128 b, 128 f] ->
                # [128 f, 128 b], col-blocked [b0 | b1] in one PSUM tile.
                ftpt = ftp.tile([128, 256], bf16, name="ftpt")
                for bh in range(2):
                    nc.tensor.transpose(
                        ftpt[:, 128 * bh : 128 * (bh + 1)],
                        ftx[bh][:, 128 * j2 : 128 * (j2 + 1)],
                        ident_s[:],
                    )
                ft = fts.tile([128, 256], bf16, name="ft")
                nc.scalar.copy(ft[:], ftpt[:])

                for bt in range(2):
                    for hh in range(2):
                        nc.tensor.matmul(
                            ps1[2 * bt + hh],
                            ft[:, 128 * bt : 128 * bt + 128],
                            w2t[:, 512 * hh : 512 * (hh + 1)],
                            start=(k == 0),
                            stop=(k == KT - 1),
                        )

    # Epilogue: copy raw fp32 h [b, 1024] to SBUF, PE-transpose to [h, b],
    # then ACT relu(h + b2') with per-partition bias, casting to bf16.
    hraw = []
    for bt in range(2):
        hr = relu_pool.tile([128, HID], f32, name=f"hraw{bt}")
        for hh in range(2):
            src = ps1[2 * bt + hh][:]
            dst = hr[:, 512 * hh : 512 * (hh + 1)]
            if bt == 0:
                nc.vector.tensor_copy(dst, src)
            else:
                nc.scalar.copy(dst, src)
        hraw.append(hr)

    hts = []
    htp_pool = ctx.enter_context(tc.tile_pool(name="htp", bufs=2, space="PSUM"))
    for p in range(4):  # k2-pairs
        tileT = htp_pool.tile([128, 512], f32, name="tileT")
        for q in range(2):  # k2 = 2p + q
            k2 = 2 * p + q
            for bt in range(2):
                nc.tensor.transpose(
                    tileT[:, 256 * q + 128 * bt : 256 * q + 128 * bt + 128],
                    hraw[bt][:, 128 * k2 : 128 * (k2 + 1)],
                    ident_f32_s[:],
                )
        for q in range(2):
            k2 = 2 * p + q
            ht = relu_pool.tile([128, 256], bf16, name=f"ht{k2}")
            nc.scalar.activation(
                ht[:],
                tileT[:, 256 * q : 256 * q + 256],
                mybir.ActivationFunctionType.Relu,
                bias=bias2_s[:, k2 : k2 + 1],
                scale=1.0,
            )
            hts.append(ht)

    # MLP2: out[b, o] per 128-row batch tile, then + b3 and DMA out.
    ps2_pool = ctx.enter_context(tc.tile_pool(name="ps2", bufs=2, space="PSUM"))
    for bt in range(2):
        p2 = ps2_pool.tile([128, OUT], f32, name="p2")
        for k2 in range(8):
            nc.tensor.matmul(
                p2[:],
                hts[k2][:, 128 * bt : 128 * bt + 128],
                w3_s[:, 256 * k2 : 256 * k2 + 256],
                start=(k2 == 0),
                stop=(k2 == 7),
            )
        ob = outs_pool.tile([128, OUT], f32, name="ob")
        nc.vector.tensor_add(ob[:], p2[:], b3rep_s[:])
        nc.sync.dma_start(out_ap[128 * bt : 128 * (bt + 1), :], ob[:])


_BUILT = {}


def _build():
    if "nc" in _BUILT:
        return _BUILT["nc"]
    nc = bacc.Bacc("TRN2", target_bir_lowering=False, debug=False)
    bf16, f32 = mybir.dt.bfloat16, mybir.dt.float32
    x_t = nc.dram_tensor("x", [BL, C, L], bf16, kind="ExternalInput")
    w2_t = nc.dram_tensor("w2", [DIN // 2, 2 * HID], bf16, kind="ExternalInput")
    w3_t = nc.dram_tensor("w3", [HID, OUT], bf16, kind="ExternalInput")
    w1rep_t = nc.dram_tensor("w1rep", [128, 4 * C], f32, kind="ExternalInput")
    bias2_t = nc.dram_tensor("bias2", [128, 8], f32, kind="ExternalInput")
    b3rep_t = nc.dram_tensor("b3rep", [128, OUT], f32, kind="ExternalInput")
    ident_t = nc.dram_tensor("ident", [128, 128], bf16, kind="ExternalInput")
    identf_t = nc.dram_tensor("identf", [128, 128], f32, kind="ExternalInput")
    out_t = nc.dram_tensor("out", [BL, OUT], f32, kind="ExternalOutput")
    with tile.TileContext(nc) as tc, ExitStack() as ctx:
        _emit(
            nc,
            tc,
            ctx,
            x_t.ap(),
            w2_t.ap(),
            w3_t.ap(),
            w1rep_t.ap(),
            bias2_t.ap(),
            b3rep_t.ap(),
            ident_t.ap(),
            identf_t.ap(),
            out_t.ap(),
        )
    nc.compile()
    _BUILT["nc"] = nc
    return nc


def _pack_weights(W1, b1, W2, b2, W3, b3):
    W1 = np.asarray(W1, np.float32)
    b1 = np.asarray(b1, np.float32)
    W2 = np.asarray(W2, np.float32)
    b2 = np.asarray(b2, np.float32)
    W3 = np.asarray(W3, np.float32)
    b3 = np.asarray(b3, np.float32)

    # Per-partition tap scalars: w1rep[p, 4c + k] = W1[c, k].
    w1rep = np.ascontiguousarray(
        np.broadcast_to(W1.reshape(1, 4 * C), (128, 4 * C))
    ).astype(np.float32)

    # Fold conv bias through W2: b2' = b2 + b1 @ sum_f W2[c*F+f, :].
    b2p = b2 + b1 @ W2.reshape(C, F, HID).sum(axis=1)
    bias2 = np.ascontiguousarray(b2p.reshape(8, 128).T).astype(np.float32)

    b3rep = np.ascontiguousarray(np.broadcast_to(b3, (128, OUT))).astype(np.float32)
    ident = np.eye(128, dtype=BF16)
    # Pack W2 so each DMA partition-row carries a contiguous 4KB k-pair:
    # packed[g*128 + p, :] = [W2[256g + p, :] | W2[256g + 128 + p, :]].
    w2b = W2.astype(BF16)
    w2packed = np.ascontiguousarray(
        w2b.reshape(DIN // 256, 2, 128, HID).swapaxes(1, 2).reshape(DIN // 2, 2 * HID)
    )
    return dict(
        w2=w2packed,
        w3=np.ascontiguousarray(W3.astype(BF16)),
        w1rep=w1rep,
        bias2=bias2,
        b3rep=b3rep,
        ident=ident,
        identf=np.eye(128, dtype=np.float32),
    )


def kernel(x, W1, b1, W2, b2, W3, b3, _trace=False):
    x = np.asarray(x, np.float32)
    # Deinterleave conv taps (x_re[b, c, k*F + f] = x[b, c, 4f + k]) and cast
    # to bf16 — the device pipeline consumes x as bf16 either way.
    x = np.ascontiguousarray(
        x.reshape(B, C, F, STEP).transpose(0, 1, 3, 2).reshape(B, C, L).astype(BF16)
    )
    nc = _build()
    shared = _pack_weights(W1, b1, W2, b2, W3, b3)
    in_maps = [dict(shared, x=x[i * BL : (i + 1) * BL]) for i in range(NCORES)]
    res = run_bass_kernel_spmd(nc, in_maps, list(range(NCORES)), trace=_trace)
    out = np.concatenate([res.results[i]["out"] for i in range(NCORES)], axis=0)
    out = out.reshape(B, 1, OUT)
    if _trace:
        kernel.last_results = res
    return out
